# revision 1
# baseline (speedup 1.0000x reference)
"""Trainium2 Bass kernel for windowed attention with relative-position bias.

Problem (hardcoded shapes):
  x        [16, 1024, 256] f32
  w_qkv    [256, 768]      f32
  w_proj   [256, 256]      f32
  b_proj   [256]           f32
  bias_table [3969, 8]     f32
  out      [16, 1024, 256] f32

Sharding: data-parallel over batch B=16 across 8 cores (2 batches/core).
Weights / bias replicated.

Per-core device algorithm (per batch b in 0..1, heads H=8, D=32, N=1024):
  qkT   = w_qk.T @ x^T            [512, 1024]  (q rows scaled by D^-0.5)
  V     = x @ w_v                 [1024, 256]  natural layout (+ ones col per head)
  per head h (processed in pairs hp = (2hp, 2hp+1)):
    S^T[m,n]  = sum_d kT[d,m] qT[d,n]           (K=32 row-packed matmuls)
    S^T      += biasT_h  (identity matmul)  OR  P = exp(S^T) * expB_h (DVE)
    P^T       = exp(S^T)                        (ACT, psum -> sbuf bf16)
    O^T_aug   = V_aug.T @ P^T  accumulated over m-chunks
                (M=33 col-packed: head pair at psum partitions 0/64;
                 row 32/96 = softmax denominators via the ones column)
  normalize: recip of denominators, broadcast via PE outer-product matmul,
             multiply during the psum->sbuf compaction copy.
  y = O'^T.T @ w_proj + b_proj    (K=32 row-packed per head) -> DMA out
"""

import numpy as np
import ml_dtypes

import concourse.bass as bass
import concourse.mybir as mybir
import concourse.tile as tile
from concourse import bacc
from concourse.bass_utils import run_bass_kernel_spmd

BF16 = mybir.dt.bfloat16
F32 = mybir.dt.float32

B, N, C = 16, 1024, 256
H, D = 8, 32
SCALE = D ** -0.5
H_GRID = W_GRID = 32
TABLE_SIZE = (2 * H_GRID - 1) * (2 * W_GRID - 1)  # 3969
N_CORES = 8
B_PER_CORE = B // N_CORES  # 2

# m-tiles whose bias is added on the PE (identity matmul on S^T in PSUM);
# the rest multiply exp(bias) on the DVE after the exp.  Tunable balance knob.
PE_BIAS_TILES = frozenset()

_nbf = ml_dtypes.bfloat16


def _relative_index():
    ys, xs = np.meshgrid(np.arange(H_GRID), np.arange(W_GRID), indexing="ij")
    grid = np.stack([ys.ravel(), xs.ravel()])
    rel = grid[:, :, None] - grid[:, None, :]
    rel[0] += H_GRID - 1
    rel[1] += W_GRID - 1
    rel[0] *= 2 * W_GRID - 1
    idx = rel.sum(0).ravel()
    return np.clip(idx, 0, TABLE_SIZE - 1)


def build_nc():
    import os
    PHASES = int(os.environ.get("K_PHASES", "4"))  # 1=A,2=+S/exp,3=+AV/drain,4=+proj
    REPEAT = int(os.environ.get("K_REPEAT", "1"))   # repeat whole body (timing)
    nc = bacc.Bacc("TRN2", target_bir_lowering=False, debug=False,
                   num_devices=N_CORES)

    xt = nc.dram_tensor("xt", [B_PER_CORE, C, N], BF16, kind="ExternalInput").ap()
    wqk = nc.dram_tensor("wqk", [C, 512], BF16, kind="ExternalInput").ap()
    wv = nc.dram_tensor("wv", [C, 256], BF16, kind="ExternalInput").ap()
    wproj = nc.dram_tensor("wproj", [C, 256], BF16, kind="ExternalInput").ap()
    bproj = nc.dram_tensor("bproj", [128, 256], F32, kind="ExternalInput").ap()
    # bmix[h, m*128:(m+1)*128, :] = biasT rows (PE tiles) or exp(biasT) rows (DVE tiles)
    bmix = nc.dram_tensor("bmix", [H, N, N], BF16, kind="ExternalInput").ap()
    ident = nc.dram_tensor("ident", [128, 128], BF16, kind="ExternalInput").ap()
    # ebc rows 0,32,64,96 are all-ones, everything else zero (broadcast lhsT)
    ebc = nc.dram_tensor("ebc", [128, 32], BF16, kind="ExternalInput").ap()
    y = nc.dram_tensor("y", [B_PER_CORE, N, C], F32, kind="ExternalOutput").ap()

    from contextlib import ExitStack
    with tile.TileContext(nc) as tc, ExitStack() as ctx:
        consts = ctx.enter_context(tc.tile_pool(name="consts", bufs=1))
        persist = ctx.enter_context(tc.tile_pool(name="persist", bufs=1))
        xt_pool = ctx.enter_context(tc.tile_pool(name="xt", bufs=2))
        bias_pool = ctx.enter_context(tc.tile_pool(name="bias", bufs=2))
        pt_pool = ctx.enter_context(tc.tile_pool(name="pt", bufs=4))
        rec_pool = ctx.enter_context(tc.tile_pool(name="rec", bufs=2))
        ysb_pool = ctx.enter_context(tc.tile_pool(name="ysb", bufs=3))
        s_psum = ctx.enter_context(tc.tile_pool(name="spsum", bufs=3, space="PSUM"))
        av_psum = ctx.enter_context(tc.tile_pool(name="avpsum", bufs=1, space="PSUM"))

        # ---- inputs first: xT for both batches, then weights ----
        xt_tiles = []
        for b in range(B_PER_CORE):
            xt_sb = xt_pool.tile([128, 2, N], BF16, name=f"xt{b}")
            nc.sync.dma_start(xt_sb[:],
                              xt[b].rearrange("(kc p) n -> p kc n", p=128))
            xt_tiles.append(xt_sb)
        # ---- constants into SBUF ----
        wqk_sb = consts.tile([128, 2, 512], BF16)
        nc.sync.dma_start(wqk_sb[:], wqk.rearrange("(kc p) m -> p kc m", p=128))
        wv_sb = consts.tile([128, 2, 256], BF16)
        nc.sync.dma_start(wv_sb[:], wv.rearrange("(kc p) m -> p kc m", p=128))
        wproj_sb = consts.tile([128, 2, 256], BF16)
        nc.sync.dma_start(wproj_sb[:], wproj.rearrange("(kc p) c -> p kc c", p=128))
        bproj2_sb = consts.tile([128, 512], F32)
        nc.sync.dma_start(bproj2_sb[:, 0:256], bproj)
        nc.sync.dma_start(bproj2_sb[:, 256:512], bproj)
        ident_sb = consts.tile([128, 128], BF16)
        nc.sync.dma_start(ident_sb[:], ident)
        ebc_sb = consts.tile([128, 32], BF16)
        nc.sync.dma_start(ebc_sb[:], ebc)
        # warm the ACT exp table while phase A runs
        actwarm = consts.tile([128, 8], F32)
        nc.scalar.activation(actwarm[:], ident_sb[:, 0:8],
                             mybir.ActivationFunctionType.Exp)

        # persistent per-batch tensors
        qk_sb = persist.tile([128, B_PER_CORE, 4, N], BF16)  # [p, b, mtile, n]
        v_sb = persist.tile([128, B_PER_CORE, 8, H * 64], BF16)  # [p, b, mchunk, h*64+c]
        ot_sb = persist.tile([128, B_PER_CORE, 4, N], BF16)  # [p, b, hp, n]
        ot_remap = persist.tile([128, B_PER_CORE, 2, N], BF16)  # [inner%128, b, kc, n]
        # denominator rows DMA-packed at partitions 0/32/64/96 per (b, kc)
        dpack = persist.tile([128, B_PER_CORE, 2, N], BF16)
        # V_aug head slots are 64 wide: cols 0-31 = V_h, col 32 = ones,
        # cols 33-63 = zero (AV writes full 64-row halves).  These memsets run
        # on the (otherwise idle) GPSIMD engine, off the DVE critical path.
        nc.gpsimd.memset(v_sb[:], 0.0)
        ones_view = v_sb.rearrange("p b m (h c) -> p b m h c", c=64)[:, :, :, :, 32:33]
        nc.gpsimd.memset(ones_view, 1.0)
        nc.gpsimd.memset(dpack[:], 0.0)

        for _rep in range(REPEAT):
            # ---- Phase A: qkT and V (emitted per batch; b1 is emitted after
            # the first attention block so the ACT pipeline starts early) ----
            def phase_a_qk(b):
                xt_sb = xt_tiles[b]
                # qkT: lhsT = wqk [256, 512] chunks, rhs = xT -> out [512, 1024]
                for mt in (0, 2, 1, 3):
                    ps = s_psum.tile([128, 1024], F32, tag="sps", name="ps")
                    for nchk in range(2):
                        for kc in range(2):
                            nc.tensor.matmul(
                                ps[:, nchk * 512:(nchk + 1) * 512],
                                lhsT=wqk_sb[:, kc, mt * 128:(mt + 1) * 128],
                                rhs=xt_sb[:, kc, nchk * 512:(nchk + 1) * 512],
                                start=(kc == 0), stop=(kc == 1),
                            )
                    nc.vector.tensor_copy(qk_sb[:, b, mt, :], ps[:])

            def phase_a_v(b):
                xt_sb = xt_tiles[b]
                # V: lhsT = xT chunks [128, ntile], rhs = wv -> out [ntile, 256]
                for g in range(2):  # groups of 4 n-tiles
                    vp = s_psum.tile([128, 1024], F32, tag="sps", name="vp")
                    for nt in range(4):
                        for kc in range(2):
                            nc.tensor.matmul(
                                vp[:, nt * 256:(nt + 1) * 256],
                                lhsT=xt_sb[:, kc, (4 * g + nt) * 128:(4 * g + nt + 1) * 128],
                                rhs=wv_sb[:, kc, :],
                                start=(kc == 0), stop=(kc == 1),
                            )
                    vsrc = vp.rearrange("p (nt h c) -> p nt h c", nt=4, h=8)
                    vdst = v_sb.rearrange("p b m (h c) -> p b m h c", c=64)[
                        :, b, 4 * g:4 * g + 4, :, 0:32]
                    nc.vector.tensor_copy(vdst, vsrc)

            def attention_block(hp, b, bias_tiles, after_m=None):
                h0, h1 = 2 * hp, 2 * hp + 1
                t = h0 // 4
                av = av_psum.tile([128, 1024], F32, name=f"av{b}", tag="avpsum")
                for m in range(8):
                    pe_bias = m in PE_BIAS_TILES
                    pair = ((0, h0), (1, h1))
                    sp = {}
                    # QK^T for both heads back-to-back: the two K=32 matmuls
                    # per nchk sit in distinct PE row groups -> concurrent.
                    for nchk in range(2):
                        sl = slice(nchk * 512, (nchk + 1) * 512)
                        for hi, h in pair:
                            bp = 32 * (h % 4)
                            if nchk == 0:
                                sp[hi] = s_psum.tile([128, 1024], F32,
                                                     tag="sps", name=f"sp{hi}")
                            nc.tensor.matmul(
                                sp[hi][:, sl],
                                lhsT=qk_sb[bp:bp + 32, b, 2 + t, m * 128:(m + 1) * 128],
                                rhs=qk_sb[bp:bp + 32, b, t, sl],
                                start=True, stop=not pe_bias,
                                tile_position=(bp, 0),
                            )
                    if pe_bias:
                        for hi, h in pair:
                            for nchk in range(2):
                                sl = slice(nchk * 512, (nchk + 1) * 512)
                                nc.tensor.matmul(
                                    sp[hi][:, sl], lhsT=ident_sb[:],
                                    rhs=bias_tiles[(hi, m)][:, sl],
                                    start=False, stop=True,
                                )
                    pt = {}
                    for hi, h in pair:
                        pt[hi] = pt_pool.tile([128, 1024], BF16,
                                              tag=f"pt{hi}", name=f"pt{hi}")
                        if pe_bias:
                            nc.scalar.activation(
                                pt[hi][:], sp[hi][:],
                                mybir.ActivationFunctionType.Exp)
                        else:
                            praw = pt_pool.tile([128, 1024], BF16,
                                                tag=f"praw{hi}", name=f"praw{hi}")
                            nc.scalar.activation(
                                praw[:], sp[hi][:],
                                mybir.ActivationFunctionType.Exp)
                            nc.vector.tensor_mul(
                                out=pt[hi][:], in0=praw[:],
                                in1=bias_tiles[(hi, m)][:])
                    if after_m and ("pre_av", m) in after_m:
                        after_m[("pre_av", m)]()
                    # AV accumulate: lhsT = V_aug head slot [128, 64];
                    # hi0/hi1 target disjoint col groups -> concurrent.
                    for nchk in (range(2) if PHASES >= 3 else range(0)):
                        sl = slice(nchk * 512, (nchk + 1) * 512)
                        for hi, h in pair:
                            po = 64 * hi
                            nc.tensor.matmul(
                                av[po:po + 64, sl],
                                lhsT=v_sb[:, b, m, h * 64:(h + 1) * 64],
                                rhs=pt[hi][:, sl],
                                start=(m == 0), stop=(m == 7),
                                tile_position=(0, po),
                                skip_group_check=True,
                            )
                    if after_m and m in after_m:
                        after_m[m]()
                if PHASES < 3:
                    return
                # evacuate unnormalized O^T (denominator rows at partitions
                # 32 / 96 ride along); then remap O rows and denominators.
                nc.vector.tensor_copy(ot_sb[:, b, hp, :], av[:])
                if PHASES >= 4:
                    for hi in range(2):
                        h = 2 * hp + hi
                        nc.sync.dma_start(
                            ot_remap[(32 * h) % 128:(32 * h) % 128 + 32,
                                     b, h // 4, :],
                            ot_sb[64 * hi:64 * hi + 32, b, hp, :])
                        nc.sync.dma_start(
                            dpack[(32 * h) % 128:(32 * h) % 128 + 1,
                                  b, h // 4, :],
                            ot_sb[32 + 64 * hi:33 + 64 * hi, b, hp, :])

            def normalize(b, kc):
                # broadcast denominators to their 32-row head blocks, then
                # reciprocal and in-place normalize of ot_remap[:, b, kc, :].
                rp = s_psum.tile([128, 1024], F32, tag="sps", name="rp")
                for k in range(4):
                    for nchk in range(2):
                        sl = slice(nchk * 512, (nchk + 1) * 512)
                        nc.tensor.matmul(
                            rp[32 * k:32 * k + 32, sl],
                            lhsT=ebc_sb[32 * k:32 * k + 32, :],
                            rhs=dpack[32 * k:32 * k + 32, b, kc, sl],
                            start=True, stop=True,
                            tile_position=(32 * k, 32 * k),
                        )
                rsb = rec_pool.tile([128, 1024], F32, tag="rsb", name="rsb")
                nc.vector.reciprocal(rsb[:], rp[:])
                nc.vector.tensor_mul(out=ot_remap[:, b, kc, :],
                                     in0=ot_remap[:, b, kc, :], in1=rsb[:])

            def phase_c(b):
                # output projection, n-tiles in pairs
                y_re = y[b].rearrange("(g p) c -> p g c", p=128)
                for ntp in range(4):
                    ysb = ysb_pool.tile([128, 2, 256], F32, name="ysb")
                    if PHASES >= 4:
                        yp = av_psum.tile([128, 512], F32, tag="avpsum",
                                          name="yp")
                        for sub in range(2):
                            nt = 2 * ntp + sub
                            for kc in range(2):
                                nc.tensor.matmul(
                                    yp[:, sub * 256:(sub + 1) * 256],
                                    lhsT=ot_remap[:, b, kc, nt * 128:(nt + 1) * 128],
                                    rhs=wproj_sb[:, kc, :],
                                    start=(kc == 0), stop=(kc == 1),
                                    skip_group_check=True,
                                )
                        nc.vector.tensor_add(
                            out=ysb.rearrange("p g c -> p (g c)"), in0=yp[:],
                            in1=bproj2_sb[:])
                    else:
                        nc.vector.tensor_copy(
                            ysb.rearrange("p g c -> p (g c)"),
                            qk_sb[:, b, 0, 0:512])
                    nc.sync.dma_start(y_re[:, 2 * ntp:2 * ntp + 2, :], ysb[:])

            phase_a_qk(0)
            if PHASES < 2:
                phase_a_v(0)
                phase_a_qk(1)
                phase_a_v(1)
                for b in range(B_PER_CORE):
                    phase_c(b)
            for hp in (range(4) if PHASES >= 2 else range(0)):
                bias_tiles = {}
                for hi, h in ((0, 2 * hp), (1, 2 * hp + 1)):
                    for m in range(8):
                        btile = bias_pool.tile([128, N], BF16,
                                               tag=f"bias_{hi}_{m}",
                                               name=f"bias_{hi}_{m}")
                        nc.sync.dma_start(
                            btile[:], bmix[h, m * 128:(m + 1) * 128, :])
                        bias_tiles[(hi, m)] = btile
                for b in range(B_PER_CORE):
                    if hp == 0 and b == 0:
                        # spread the rest of phase A through the first
                        # (ACT-bound) attention block so the PE fills its
                        # idle slots with it instead of stalling the ACT.
                        attention_block(hp, b, bias_tiles, after_m={
                            ("pre_av", 0): lambda: phase_a_v(0),
                            2: lambda: phase_a_qk(1),
                            4: lambda: phase_a_v(1),
                        })
                    else:
                        attention_block(hp, b, bias_tiles)
                    # each batch's normalize/projection runs in the shadow of
                    # the other batch's ACT-bound attention block.
                    if PHASES >= 4 and hp % 2 == 1:
                        normalize(b, hp // 2)
                        if hp == 3:
                            phase_c(b)

    nc.compile()
    return nc


_NC_CACHE = None


def _get_nc():
    global _NC_CACHE
    if _NC_CACHE is None:
        _NC_CACHE = build_nc()
    return _NC_CACHE


def _host_prep(x, w_qkv, w_proj, b_proj, bias_table):
    idx = _relative_index()
    bias = bias_table.astype(np.float32)[idx].reshape(N, N, H)  # [n, m, h]
    biasT = np.ascontiguousarray(np.transpose(bias, (2, 1, 0)))  # [h, m, n]
    bmix = np.empty((H, N, N), dtype=_nbf)
    for m in range(8):
        rows = slice(m * 128, (m + 1) * 128)
        if m in PE_BIAS_TILES:
            bmix[:, rows, :] = biasT[:, rows, :].astype(_nbf)
        else:
            bmix[:, rows, :] = np.exp(biasT[:, rows, :]).astype(_nbf)

    xt = np.ascontiguousarray(np.transpose(x, (0, 2, 1))).astype(_nbf)  # [B, C, N]
    w_qk = np.concatenate(
        [w_qkv[:, :256] * SCALE, w_qkv[:, 256:512]], axis=1).astype(_nbf)
    w_v = w_qkv[:, 512:].astype(_nbf)
    wproj_arr = w_proj.astype(_nbf)
    bproj_rep = np.ascontiguousarray(
        np.broadcast_to(b_proj.astype(np.float32), (128, 256)))
    ident = np.eye(128, dtype=_nbf)
    ebc = np.zeros((128, 32), dtype=_nbf)
    for p in (0, 32, 64, 96):
        ebc[p, :] = 1.0
    return xt, w_qk, w_v, wproj_arr, bproj_rep, bmix, ident, ebc


def kernel(x, w_qkv, w_proj, b_proj, bias_table):
    x = np.asarray(x, dtype=np.float32)
    w_qkv = np.asarray(w_qkv, dtype=np.float32)
    w_proj = np.asarray(w_proj, dtype=np.float32)
    b_proj = np.asarray(b_proj, dtype=np.float32)
    bias_table = np.asarray(bias_table, dtype=np.float32)

    xt, w_qk, w_v, wproj_arr, bproj_rep, bmix, ident, ebc = _host_prep(
        x, w_qkv, w_proj, b_proj, bias_table)

    nc = _get_nc()
    in_maps = []
    for c in range(N_CORES):
        in_maps.append({
            "xt": xt[B_PER_CORE * c:B_PER_CORE * (c + 1)],
            "wqk": w_qk, "wv": w_v, "wproj": wproj_arr, "bproj": bproj_rep,
            "bmix": bmix, "ident": ident, "ebc": ebc,
        })
    res = run_bass_kernel_spmd(nc, in_maps, core_ids=list(range(N_CORES)))
    out = np.concatenate([res.results[c]["y"] for c in range(N_CORES)], axis=0)
    return out.astype(np.float32)


if __name__ == "__main__":
    rng = np.random.default_rng(0)
    inputs = {
        "x": rng.standard_normal((B, N, C), dtype=np.float32),
        "w_qkv": (rng.standard_normal((C, 3 * 256), dtype=np.float32) * C ** -0.5),
        "w_proj": (rng.standard_normal((256, C), dtype=np.float32) * 256 ** -0.5),
        "b_proj": np.zeros((C,), dtype=np.float32),
        "bias_table": (rng.standard_normal((TABLE_SIZE, H), dtype=np.float32) * 0.02),
    }
    out = kernel(**inputs)
    print("kernel output", out.shape, out.dtype)

